# revision 20
# baseline (speedup 1.0000x reference)
"""Trainium2 Bass kernel for nn_Adjacency (gnn_message_passing).

Computation (per graph g in 0..2):
    D[i,j] = ||nv[i] - nv[j]||  masked by adj_g   (64x64, tiny)
    out_g  = relu(relu(vec(D) @ Wg1) @ Wg2)       (two 4096x4096 mat-vecs)

Sharding across 8 NeuronCores (tensor-parallel on the mat-vecs):
    core k holds Wg1[:, 512k:512(k+1)]  (columns)  and
                 Wg2[512k:512(k+1), :]  (rows).
    Each core computes h_k = relu(v @ Wg1_shard) locally (ReLU is
    elementwise in the sharded dim), then partial_k = h_k @ Wg2_shard.
    The host sums the 8 partials and applies the final ReLU (12K elts).
    The distance stage is replicated on every core (it is ~1M MACs).

Weights are cast to fp16 on the host: halves the HBM traffic (the
memory-bound term) and avoids the PE's 2-pass fp32 LOW_HIGH matmul
mode. The distance stage stays fp32; the matvec operands (v, h) are
rounded to fp16. End-to-end error vs the fp32 reference is ~5e-4
relative to the output scale.

Per-core traffic: 6 weight shards x 4 MiB = 24 MiB -> memory-bound at
~360 GB/s per-core HBM bandwidth.
"""

import numpy as np

N = 64
F = 256
U = N * N          # 4096
NCORES = 8
SH = U // NCORES   # 512

_CACHE = {}


def _build_nc():
    """Build + compile the (SPMD, per-core) Bass program once per process."""
    import concourse.mybir as mybir
    import concourse.tile as tile
    from concourse import bacc
    from concourse.masks import make_identity

    FP = mybir.dt.float32
    F16 = mybir.dt.float16
    AF = mybir.ActivationFunctionType

    nc = bacc.Bacc(
        "TRN2",
        target_bir_lowering=False,
        debug=False,
        enable_asserts=False,
        num_devices=NCORES,
    )

    nv_d = nc.dram_tensor("nv", [N, F], FP, kind="ExternalInput")
    # adj pre-reshaped on host to [3, 32, 128] (row-major flatten of 64x64)
    adj_d = nc.dram_tensor("adj", [3, 32, 128], FP, kind="ExternalInput")
    # w1_g shard pretiled on host to [4, 128, 4096] fp16 quarter-shards:
    #   [q, p, 512u + f] = Wg1[128(8q+u) + p, 512k + f]          (k-chunk 8q+u)
    w1_d = [
        nc.dram_tensor(f"w1_{g}", [4, 128, 4096], F16, kind="ExternalInput")
        for g in range(3)
    ]
    # w2_g shard pretiled on host to [4, 128, 4096] fp16 (four 1 MiB
    # quarter-shards so the tail graph's layer-2 compute overlaps its
    # weight stream):
    #   [t, p, n] = Wg2[512k + 128t + p, n]                      (k-chunk t)
    w2_d = [
        nc.dram_tensor(f"w2_{g}", [4, 128, 4096], F16, kind="ExternalInput")
        for g in range(3)
    ]
    out_d = nc.dram_tensor("out", [3, U], FP, kind="ExternalOutput")

    with tile.TileContext(nc) as tc:
        with (
            tc.tile_pool(name="const", bufs=1) as constp,
            tc.tile_pool(name="w1p", bufs=8) as w1p,
            tc.tile_pool(name="w2p", bufs=8) as w2p,
            tc.tile_pool(name="hbuf", bufs=2) as hbufp,
            tc.tile_pool(name="obuf", bufs=2) as obufp,
            tc.tile_pool(name="vbuf", bufs=3) as vbufp,
            tc.tile_pool(name="ps_small", bufs=2, space="PSUM") as ps_small,
            tc.tile_pool(name="ps_h", bufs=2, space="PSUM") as ps_h,
            tc.tile_pool(name="ps_o", bufs=4, space="PSUM") as ps_o,
        ):
            # nv/adj first, then the weight stream, all via SWDGE (gpsimd):
            # the Q7 starts draining DMAs during the kernel-entry barrier,
            # several us before the HWDGE rings issue their first transfer.
            # Quarter-granularity tiles let the consuming matmuls chase the
            # stream instead of waiting on whole-shard DMAs.
            nv_sb = constp.tile([N, F], FP)
            nc.gpsimd.dma_start(nv_sb[:], nv_d[:])
            adj_sb = constp.tile([32, 3 * 128], FP)
            nc.gpsimd.dma_start(
                adj_sb[:].rearrange("q (g t) -> q g t", g=3),
                adj_d[:].rearrange("g q t -> q g t"),
            )
            w1ts, w2ts = [], []
            for g in range(3):
                quads1, quads2 = [], []
                for s in range(4):
                    w1t = w1p.tile([128, 4096], F16, tag="w1")
                    nc.gpsimd.dma_start(w1t[:], w1_d[g][s])
                    quads1.append(w1t)
                for s in range(4):
                    w2t = w2p.tile([128, 4096], F16, tag="w2")
                    nc.gpsimd.dma_start(w2t[:], w2_d[g][s])
                    quads2.append(w2t)
                w1ts.append(quads1)
                w2ts.append(quads2)

            ident = constp.tile([128, 128], FP)
            make_identity(nc, ident[:])
            ones = constp.tile([128, 64], FP)
            nc.vector.memset(ones[:], 1.0)

            # nvT [128, 128]: chunk c (=feature block) at [:, 64c:64c+64],
            # nvT[p, 64c+j] = nv[j, 128c+p]
            nvT = constp.tile([128, 128], FP)
            for c in range(2):
                pst = ps_small.tile([128, 64], FP, tag="small")
                nc.tensor.transpose(
                    pst[:], nv_sb[:, 128 * c : 128 * (c + 1)], ident[0:64, 0:64]
                )
                nc.scalar.copy(nvT[:, 64 * c : 64 * (c + 1)], pst[:])

            # squared row-norms as a row vector, scaled by -0.5:
            # nh[0, j] = -0.5 * sum_f nv[j, f]^2
            nvTsq = constp.tile([128, 128], FP)
            nc.scalar.activation(nvTsq[:], nvT[:], AF.Square)
            psn = ps_small.tile([1, 64], FP, tag="small")
            nc.tensor.matmul(psn[:], ones[:, 0:1], nvTsq[:, 0:64], start=True, stop=False)
            nc.tensor.matmul(psn[:], ones[:, 0:1], nvTsq[:, 64:128], start=False, stop=True)
            nh = constp.tile([1, 64], FP)
            nc.scalar.mul(nh[:], psn[:], -0.5)

            # Distance stage per graph: psA[q, 64e+j] = G[2q+e, j]
            # - 0.5 n[2q+e] - 0.5 n[j]; dist^2 = -2 * psA.
            # The [32, 128] layout makes vec(D) = rows, so a single PE
            # transpose yields the matvec operand in k-chunk-column form.
            vcols = []
            for g in range(3):
                psA = ps_small.tile([32, 128], FP, tag="small")
                for e in range(2):
                    oslc = psA[:, 64 * e : 64 * (e + 1)]
                    nc.tensor.matmul(
                        oslc, nvT[:, e:64:2], nvT[:, 0:64], start=True, stop=False
                    )
                    nc.tensor.matmul(
                        oslc, nvT[:, 64 + e : 128 : 2], nvT[:, 64:128],
                        start=False, stop=False,
                    )
                    nc.tensor.matmul(
                        oslc, nh[0:1, e:64:2], ones[0:1, 0:64], start=False, stop=False
                    )
                    nc.tensor.matmul(
                        oslc, ones[0:1, 0:32], nh[0:1, 0:64], start=False, stop=True
                    )
                dist = vbufp.tile([32, 128], FP, tag="dist")
                nc.scalar.activation(dist[:], psA[:], AF.Relu, scale=-2.0)
                nc.scalar.activation(dist[:], dist[:], AF.Sqrt)
                nc.vector.tensor_mul(
                    dist[:], dist[:], adj_sb[:, 128 * g : 128 * (g + 1)]
                )
                vps = ps_small.tile([128, 32], FP, tag="small")
                nc.tensor.transpose(vps[:], dist[:], ident[0:32, 0:32])
                vcol = vbufp.tile([128, 32], F16, tag="vcol")  # cast to fp16
                nc.vector.tensor_copy(vcol[:], vps[:])
                vcols.append(vcol)

            for g in range(3):
                # Layer 1: h = relu(v @ W1_shard), K=4096 in 32 chunks of
                # 128; the fp16 v column is the stationary operand, the
                # fp16 weight chunk [128, 512] streams through.
                psh = ps_h.tile([1, SH], FP, tag="psh")
                for c in range(32):
                    nc.tensor.matmul(
                        psh[:],
                        vcols[g][:, c : c + 1],
                        w1ts[g][c // 8][:, 512 * (c % 8) : 512 * (c % 8 + 1)],
                        start=(c == 0),
                        stop=(c == 31),
                    )
                h_row = hbufp.tile([1, SH], FP, tag="hrow")
                nc.scalar.activation(h_row[:], psh[:], AF.Relu)
                # h [1,512] -> column-chunk form [128, 4], cast to fp16
                hps = ps_small.tile([128, 4], FP, tag="small")
                for c4 in range(4):
                    nc.tensor.transpose(
                        hps[:, c4 : c4 + 1],
                        h_row[0:1, 128 * c4 : 128 * (c4 + 1)],
                        ident[0:1, 0:1],
                    )
                h_col = hbufp.tile([128, 4], F16, tag="hcol")
                nc.vector.tensor_copy(h_col[:], hps[:])

                # Layer 2: partial = h_shard @ W2_shard, K=512 (4 chunks),
                # N=4096 (8 psum banks).
                out_row = obufp.tile([1, U], FP, tag="orow")
                if g < 2:
                    # mid-stream: j-outer, 4 rotating psum slots
                    for j in range(8):
                        pso = ps_o.tile([1, 512], FP, tag="pso")
                        for t in range(4):
                            nc.tensor.matmul(
                                pso[:],
                                h_col[:, t : t + 1],
                                w2ts[g][t][:, 512 * j : 512 * (j + 1)],
                                start=(t == 0),
                                stop=(t == 3),
                            )
                        nc.vector.tensor_copy(
                            out_row[0:1, 512 * j : 512 * (j + 1)], pso[:]
                        )
                else:
                    # kernel tail: k-outer so each weight quarter is fully
                    # consumed as it lands; all 8 banks (4 from ps_o, 2
                    # borrowed from each of ps_small/ps_h) accumulate.
                    psos = (
                        [ps_o.tile([1, 512], FP, tag="pso", name=f"pso_{i}") for i in range(4)]
                        + [ps_small.tile([1, 512], FP, tag="small", name=f"psos_{i}") for i in range(2)]
                        + [ps_h.tile([1, 512], FP, tag="psh", name=f"psoh_{i}") for i in range(2)]
                    )
                    for t in range(4):
                        for j in range(8):
                            nc.tensor.matmul(
                                psos[j][:],
                                h_col[:, t : t + 1],
                                w2ts[g][t][:, 512 * j : 512 * (j + 1)],
                                start=(t == 0),
                                stop=(t == 3),
                            )
                    for j in range(8):
                        eng = nc.vector.tensor_copy if j % 2 == 0 else nc.scalar.copy
                        eng(out_row[0:1, 512 * j : 512 * (j + 1)], psos[j][:])
                nc.scalar.dma_start(out_d[g : g + 1, :], out_row[:])

    nc.compile()
    return nc


def get_nc():
    if "nc" not in _CACHE:
        _CACHE["nc"] = _build_nc()
    return _CACHE["nc"]


def prep_in_maps(inputs):
    """Host-side sharding: per-core input dicts (weights pre-tiled, fp16)."""
    nv = np.ascontiguousarray(np.asarray(inputs["node_vec"], np.float32).reshape(N, F))
    adj = np.ascontiguousarray(
        np.stack(
            [np.asarray(inputs[f"adj{g}"], np.float32).reshape(32, 128) for g in range(3)]
        )
    )
    W1 = [np.asarray(inputs[k], np.float32) for k in ("w0_1", "w1_1", "w2_1")]
    W2 = [np.asarray(inputs[k], np.float32) for k in ("w0_2", "w1_2", "w2_2")]
    in_maps = []
    for k in range(NCORES):
        m = {"nv": nv, "adj": adj}
        for g in range(3):
            w1s = W1[g][:, SH * k : SH * (k + 1)].astype(np.float16)  # [4096, 512]
            m[f"w1_{g}"] = np.ascontiguousarray(
                w1s.reshape(4, 8, 128, 512).transpose(0, 2, 1, 3)
            ).reshape(4, 128, 4096)
            w2s = W2[g][SH * k : SH * (k + 1), :].astype(np.float16)  # [512, 4096]
            m[f"w2_{g}"] = np.ascontiguousarray(w2s).reshape(4, 128, 4096)
        in_maps.append(m)
    return in_maps


def run_sharded(inputs, **run_kwargs):
    """Compile (cached), shard, run on 8 cores; returns BassKernelResults."""
    import concourse.bass_utils as bass_utils

    nc = get_nc()
    in_maps = prep_in_maps(inputs)
    return bass_utils.run_bass_kernel_spmd(
        nc, in_maps, core_ids=list(range(NCORES)), **run_kwargs
    )


def gather(results):
    """Sum per-core partials, final ReLU, reshape to 3x(64,64)."""
    tot = np.zeros((3, U), np.float32)
    for r in results:
        tot += np.asarray(r["out"], np.float32)
    out = np.maximum(tot, 0.0).reshape(3, N, N)
    return out[0], out[1], out[2]


def kernel(**inputs):
    res = run_sharded(inputs)
    return gather(res.results)


# revision 22
# speedup vs baseline: 1.1945x; 1.1945x over previous
"""Trainium2 Bass kernel for nn_Adjacency (gnn_message_passing).

Computation (per graph g in 0..2):
    D[i,j] = ||nv[i] - nv[j]||  masked by adj_g   (64x64, tiny)
    out_g  = relu(relu(vec(D) @ Wg1) @ Wg2)       (two 4096x4096 mat-vecs)

Sharding across 8 NeuronCores (tensor-parallel on the mat-vecs):
    core k holds Wg1[:, 512k:512(k+1)]  (columns)  and
                 Wg2[512k:512(k+1), :]  (rows).
    Each core computes h_k = relu(v @ Wg1_shard) locally (ReLU is
    elementwise in the sharded dim), then partial_k = h_k @ Wg2_shard.
    The host sums the 8 partials and applies the final ReLU (12K elts).
    The distance stage is replicated on every core (it is ~1M MACs).

Weights are cast to fp16 on the host: halves the HBM traffic (the
memory-bound term) and avoids the PE's 2-pass fp32 LOW_HIGH matmul
mode. The distance stage stays fp32; the matvec operands (v, h) are
rounded to fp16. End-to-end error vs the fp32 reference is ~5e-4
relative to the output scale.

Per-core traffic: 6 weight shards x 4 MiB = 24 MiB -> memory-bound at
~360 GB/s per-core HBM bandwidth.
"""

import numpy as np

N = 64
F = 256
U = N * N          # 4096
NCORES = 8
SH = U // NCORES   # 512

_CACHE = {}


def _build_nc():
    """Build + compile the (SPMD, per-core) Bass program once per process."""
    import concourse.mybir as mybir
    import concourse.tile as tile
    from concourse import bacc
    from concourse.masks import make_identity

    FP = mybir.dt.float32
    F16 = mybir.dt.float16
    AF = mybir.ActivationFunctionType

    nc = bacc.Bacc(
        "TRN2",
        target_bir_lowering=False,
        debug=False,
        enable_asserts=False,
        num_devices=NCORES,
    )

    nv_d = nc.dram_tensor("nv", [N, F], FP, kind="ExternalInput")
    # adj pre-reshaped on host to [3, 32, 128] (row-major flatten of 64x64)
    adj_d = nc.dram_tensor("adj", [3, 32, 128], FP, kind="ExternalInput")
    # w1_g shard pretiled on host to [4, 128, 4096] fp16 quarter-shards:
    #   [q, p, 512u + f] = Wg1[128(8q+u) + p, 512k + f]          (k-chunk 8q+u)
    w1_d = [
        nc.dram_tensor(f"w1_{g}", [4, 128, 4096], F16, kind="ExternalInput")
        for g in range(3)
    ]
    # w2_g shard pretiled on host to [4, 128, 4096] fp16 (four 1 MiB
    # quarter-shards so the tail graph's layer-2 compute overlaps its
    # weight stream):
    #   [t, p, n] = Wg2[512k + 128t + p, n]                      (k-chunk t)
    w2_d = [
        nc.dram_tensor(f"w2_{g}", [4, 128, 4096], F16, kind="ExternalInput")
        for g in range(3)
    ]
    out_d = nc.dram_tensor("out", [3, U], FP, kind="ExternalOutput")

    with tile.TileContext(nc) as tc:
        with (
            tc.tile_pool(name="const", bufs=1) as constp,
            tc.tile_pool(name="w1p", bufs=8) as w1p,
            tc.tile_pool(name="w2p", bufs=8) as w2p,
            tc.tile_pool(name="hbuf", bufs=2) as hbufp,
            tc.tile_pool(name="obuf", bufs=2) as obufp,
            tc.tile_pool(name="vbuf", bufs=3) as vbufp,
            tc.tile_pool(name="ps_small", bufs=2, space="PSUM") as ps_small,
            tc.tile_pool(name="ps_h", bufs=2, space="PSUM") as ps_h,
            tc.tile_pool(name="ps_o", bufs=4, space="PSUM") as ps_o,
        ):
            # nv/adj first, then the weight stream, all via SWDGE (gpsimd):
            # the Q7 starts draining DMAs during the kernel-entry barrier,
            # several us before the HWDGE rings issue their first transfer.
            # Quarter-granularity tiles let the consuming matmuls chase the
            # stream instead of waiting on whole-shard DMAs.
            nv_sb = constp.tile([N, F], FP)
            nc.gpsimd.dma_start(nv_sb[:], nv_d[:])
            adj_sb = constp.tile([32, 3 * 128], FP)
            nc.gpsimd.dma_start(
                adj_sb[:].rearrange("q (g t) -> q g t", g=3),
                adj_d[:].rearrange("g q t -> q g t"),
            )
            # Stream order: W1g0, W2g0, W1g1, W1g2, W2g1, W2g2 — both
            # remaining W1 shards land before the final W2 shards so the
            # L1(g)->h->L2(g) chain of the tail graphs overlaps the stream.
            w1ts = [[None] * 4 for _ in range(3)]
            w2ts = [[None] * 4 for _ in range(3)]

            def _load(dst, pool, dram, g, tag):
                for s in range(4):
                    t = pool.tile([128, 4096], F16, tag=tag, name=f"{tag}_{g}_{s}")
                    nc.sync.dma_start(t[:], dram[g][s])
                    dst[g][s] = t

            _load(w1ts, w1p, w1_d, 0, "w1")
            _load(w2ts, w2p, w2_d, 0, "w2")
            _load(w1ts, w1p, w1_d, 1, "w1")
            _load(w1ts, w1p, w1_d, 2, "w1")
            _load(w2ts, w2p, w2_d, 1, "w2")
            _load(w2ts, w2p, w2_d, 2, "w2")

            ident = constp.tile([128, 128], FP)
            make_identity(nc, ident[:])
            ones = constp.tile([128, 64], FP)
            nc.vector.memset(ones[:], 1.0)

            # nvT [128, 128]: chunk c (=feature block) at [:, 64c:64c+64],
            # nvT[p, 64c+j] = nv[j, 128c+p]
            nvT = constp.tile([128, 128], FP)
            for c in range(2):
                pst = ps_small.tile([128, 64], FP, tag="small")
                nc.tensor.transpose(
                    pst[:], nv_sb[:, 128 * c : 128 * (c + 1)], ident[0:64, 0:64]
                )
                nc.scalar.copy(nvT[:, 64 * c : 64 * (c + 1)], pst[:])

            # squared row-norms as a row vector, scaled by -0.5:
            # nh[0, j] = -0.5 * sum_f nv[j, f]^2
            nvTsq = constp.tile([128, 128], FP)
            nc.scalar.activation(nvTsq[:], nvT[:], AF.Square)
            psn = ps_small.tile([1, 64], FP, tag="small")
            nc.tensor.matmul(psn[:], ones[:, 0:1], nvTsq[:, 0:64], start=True, stop=False)
            nc.tensor.matmul(psn[:], ones[:, 0:1], nvTsq[:, 64:128], start=False, stop=True)
            nh = constp.tile([1, 64], FP)
            nc.scalar.mul(nh[:], psn[:], -0.5)

            # Distance stage (graph-independent): psA[q, 64e+j] = G[2q+e, j]
            # - 0.5 n[2q+e] - 0.5 n[j]; dist^2 = -2 * psA.
            # The [32, 128] layout makes vec(D) = rows, so a single PE
            # transpose yields the matvec operand in k-chunk-column form.
            psA = ps_small.tile([32, 128], FP, tag="small")
            for e in range(2):
                oslc = psA[:, 64 * e : 64 * (e + 1)]
                nc.tensor.matmul(
                    oslc, nvT[:, e:64:2], nvT[:, 0:64], start=True, stop=False
                )
                nc.tensor.matmul(
                    oslc, nvT[:, 64 + e : 128 : 2], nvT[:, 64:128],
                    start=False, stop=False,
                )
                nc.tensor.matmul(
                    oslc, nh[0:1, e:64:2], ones[0:1, 0:64], start=False, stop=False
                )
                nc.tensor.matmul(
                    oslc, ones[0:1, 0:32], nh[0:1, 0:64], start=False, stop=True
                )
            dist0 = constp.tile([32, 128], FP)
            nc.scalar.activation(dist0[:], psA[:], AF.Relu, scale=-2.0)
            nc.scalar.activation(dist0[:], dist0[:], AF.Sqrt)
            # Per graph: mask by adj, transpose, cast to fp16
            vcols = []
            for g in range(3):
                dist = vbufp.tile([32, 128], FP, tag="dist")
                nc.vector.tensor_mul(
                    dist[:], dist0[:], adj_sb[:, 128 * g : 128 * (g + 1)]
                )
                vps = ps_small.tile([128, 32], FP, tag="small")
                nc.tensor.transpose(vps[:], dist[:], ident[0:32, 0:32])
                vcol = vbufp.tile([128, 32], F16, tag="vcol")  # cast to fp16
                nc.vector.tensor_copy(vcol[:], vps[:])
                vcols.append(vcol)

            for g in range(3):
                # Layer 1: h = relu(v @ W1_shard), K=4096 in 32 chunks of
                # 128; the fp16 v column is the stationary operand, the
                # fp16 weight chunk [128, 512] streams through.
                psh = ps_h.tile([1, SH], FP, tag="psh")
                for c in range(32):
                    nc.tensor.matmul(
                        psh[:],
                        vcols[g][:, c : c + 1],
                        w1ts[g][c // 8][:, 512 * (c % 8) : 512 * (c % 8 + 1)],
                        start=(c == 0),
                        stop=(c == 31),
                    )
                h_row = hbufp.tile([1, SH], FP, tag="hrow")
                nc.scalar.activation(h_row[:], psh[:], AF.Relu)
                # h [1,512] -> column-chunk form [128, 4], cast to fp16
                hps = ps_small.tile([128, 4], FP, tag="small")
                for c4 in range(4):
                    nc.tensor.transpose(
                        hps[:, c4 : c4 + 1],
                        h_row[0:1, 128 * c4 : 128 * (c4 + 1)],
                        ident[0:1, 0:1],
                    )
                h_col = hbufp.tile([128, 4], F16, tag="hcol")
                nc.vector.tensor_copy(h_col[:], hps[:])

                # Layer 2: partial = h_shard @ W2_shard, K=512 (4 chunks),
                # N=4096 (8 psum banks).
                out_row = obufp.tile([1, U], FP, tag="orow")
                if g < 2:
                    # mid-stream: j-outer, 4 rotating psum slots
                    for j in range(8):
                        pso = ps_o.tile([1, 512], FP, tag="pso")
                        for t in range(4):
                            nc.tensor.matmul(
                                pso[:],
                                h_col[:, t : t + 1],
                                w2ts[g][t][:, 512 * j : 512 * (j + 1)],
                                start=(t == 0),
                                stop=(t == 3),
                            )
                        nc.vector.tensor_copy(
                            out_row[0:1, 512 * j : 512 * (j + 1)], pso[:]
                        )
                else:
                    # kernel tail: k-outer so each weight quarter is fully
                    # consumed as it lands; all 8 banks (4 from ps_o, 2
                    # borrowed from each of ps_small/ps_h) accumulate.
                    psos = (
                        [ps_o.tile([1, 512], FP, tag="pso", name=f"pso_{i}") for i in range(4)]
                        + [ps_small.tile([1, 512], FP, tag="small", name=f"psos_{i}") for i in range(2)]
                        + [ps_h.tile([1, 512], FP, tag="psh", name=f"psoh_{i}") for i in range(2)]
                    )
                    for t in range(4):
                        for j in range(8):
                            nc.tensor.matmul(
                                psos[j][:],
                                h_col[:, t : t + 1],
                                w2ts[g][t][:, 512 * j : 512 * (j + 1)],
                                start=(t == 0),
                                stop=(t == 3),
                            )
                    for j in range(8):
                        eng = nc.vector.tensor_copy if j % 2 == 0 else nc.scalar.copy
                        eng(out_row[0:1, 512 * j : 512 * (j + 1)], psos[j][:])
                nc.scalar.dma_start(out_d[g : g + 1, :], out_row[:])

    nc.compile()
    return nc


def get_nc():
    if "nc" not in _CACHE:
        _CACHE["nc"] = _build_nc()
    return _CACHE["nc"]


def prep_in_maps(inputs):
    """Host-side sharding: per-core input dicts (weights pre-tiled, fp16)."""
    nv = np.ascontiguousarray(np.asarray(inputs["node_vec"], np.float32).reshape(N, F))
    adj = np.ascontiguousarray(
        np.stack(
            [np.asarray(inputs[f"adj{g}"], np.float32).reshape(32, 128) for g in range(3)]
        )
    )
    W1 = [np.asarray(inputs[k], np.float32) for k in ("w0_1", "w1_1", "w2_1")]
    W2 = [np.asarray(inputs[k], np.float32) for k in ("w0_2", "w1_2", "w2_2")]
    in_maps = []
    for k in range(NCORES):
        m = {"nv": nv, "adj": adj}
        for g in range(3):
            w1s = W1[g][:, SH * k : SH * (k + 1)].astype(np.float16)  # [4096, 512]
            m[f"w1_{g}"] = np.ascontiguousarray(
                w1s.reshape(4, 8, 128, 512).transpose(0, 2, 1, 3)
            ).reshape(4, 128, 4096)
            w2s = W2[g][SH * k : SH * (k + 1), :].astype(np.float16)  # [512, 4096]
            m[f"w2_{g}"] = np.ascontiguousarray(w2s).reshape(4, 128, 4096)
        in_maps.append(m)
    return in_maps


def run_sharded(inputs, **run_kwargs):
    """Compile (cached), shard, run on 8 cores; returns BassKernelResults."""
    import concourse.bass_utils as bass_utils

    nc = get_nc()
    in_maps = prep_in_maps(inputs)
    return bass_utils.run_bass_kernel_spmd(
        nc, in_maps, core_ids=list(range(NCORES)), **run_kwargs
    )


def gather(results):
    """Sum per-core partials, final ReLU, reshape to 3x(64,64)."""
    tot = np.zeros((3, U), np.float32)
    for r in results:
        tot += np.asarray(r["out"], np.float32)
    out = np.maximum(tot, 0.0).reshape(3, N, N)
    return out[0], out[1], out[2]


def kernel(**inputs):
    res = run_sharded(inputs)
    return gather(res.results)
